# revision 5
# baseline (speedup 1.0000x reference)
"""Trainium2 Bass kernel for nn_CascadeCore_3882650436478.

Data-parallel over batch: 8 NeuronCores x 4 batches (64 tokens each).
Weights are replicated; no cross-device communication.

Per-core dataflow (all matmul contractions on the partition dim, no
on-chip transposes — host pre-packs everything K-major):

  pooledT[f,t] = pool_feats[b].T-layout @ roiT_scaled[b]   (cnt folded in)
  xT = [rnnT; pooledT]                    (16 K-tiles of (128, 64))
  hT[m] = relu(W1catT[:,m].T @ xT + b1)   (16 M-tiles: 8 bn + 8 fg)
  bn_logits = h_bnT.T @ W_bn2.T + ones.T @ b_bn2       (64, 2)
  fg_outT[mt] = W_fg2T[:,mt].T @ h_fgT + b_fg2         (3 M-tiles, padded 300->384)
  score = fg_outT.T @ fg_embT + maskadd                (64, 461)
  out = log_softmax over free dim for both branches

MLP1 runs K-outer with both halves' accumulators packed into two
PSUM bank tiles (128, 512), so compute starts as soon as the first
W1 chunk lands and overlaps the remaining chunk DMAs.

Compute dtype modes (BASS_CASCADE_DTYPE): "bf16" (both stages bf16),
"mixed" (stage-1 bf16, stage-2 f32), "f32".
"""

import os

import numpy as np
import ml_dtypes

import concourse.bass as bass
import concourse.mybir as mybir
import concourse.tile as tile
from concourse import bacc
from concourse.bass_utils import run_bass_kernel_spmd

B, S, R = 32, 16, 128
RNN = 1024
FG = 461
MIN_VALUE = -1e8

N_CORES = 8
BPC = B // N_CORES          # batches per core
T = BPC * S                 # tokens per core (64)
P = 128
KT1 = (2 * RNN) // P        # 16 K-tiles for MLP1
MT1 = (2 * RNN) // P        # 16 M-tiles (8 bn hid + 8 fg hid)
KT2 = RNN // P              # 8 K-tiles for MLP2
FGP = 384                   # fg_out features padded 300 -> 384
MT2 = FGP // P              # 3 M-tiles for fg_out
W1_CHUNKS = 4
KO_PER_CHUNK = KT1 // W1_CHUNKS

COMPUTE_DTYPE = os.environ.get("BASS_CASCADE_DTYPE", "mixed")

_BUILD_CACHE = {}


def _dts(mode):
    bf, f32 = mybir.dt.bfloat16, mybir.dt.float32
    return {"f32": (f32, f32), "mixed": (bf, f32), "bf16": (bf, bf)}[mode]


def _np_dts(mode):
    bf, f32 = ml_dtypes.bfloat16, np.float32
    return {"f32": (f32, f32), "mixed": (bf, f32), "bf16": (bf, bf)}[mode]


def _pack_kmajor(a: np.ndarray) -> np.ndarray:
    """(K, M) -> (128, K//128 * M); element (p, ko*M+m) = a[ko*128+p, m]."""
    K, M = a.shape
    assert K % P == 0
    return np.ascontiguousarray(
        a.reshape(K // P, P, M).transpose(1, 0, 2).reshape(P, (K // P) * M)
    )


def _build(mode: str):
    if mode in _BUILD_CACHE:
        return _BUILD_CACHE[mode]

    DT1, DT2 = _dts(mode)
    F32 = mybir.dt.float32
    AF = mybir.ActivationFunctionType
    ALU = mybir.AluOpType
    AX = mybir.AxisListType

    nc = bacc.Bacc(
        "TRN2", target_bir_lowering=False, debug=False, enable_asserts=False
    )

    w1t = nc.declare_dram_parameter("w1t", [P, KT1 * 2 * RNN], DT1, isOutput=False)
    xr = nc.declare_dram_parameter("xr", [P, KT2 * T], DT1, isOutput=False)
    poolf = nc.declare_dram_parameter("poolf", [P, BPC * RNN], DT1, isOutput=False)
    roit = nc.declare_dram_parameter("roit", [P, BPC * S], DT1, isOutput=False)
    w2t = nc.declare_dram_parameter("w2t", [P, KT2 * FGP], DT2, isOutput=False)
    w2bnt = nc.declare_dram_parameter("w2bnt", [P, KT2 * 2], DT2, isOutput=False)
    embt = nc.declare_dram_parameter("embt", [P, MT2 * FG], DT2, isOutput=False)
    bnb = nc.declare_dram_parameter("bnb", [1, 2], DT2, isOutput=False)
    b1 = nc.declare_dram_parameter("b1", [P, MT1], F32, isOutput=False)
    bfg2 = nc.declare_dram_parameter("bfg2", [P, MT2], F32, isOutput=False)
    maskadd = nc.declare_dram_parameter("maskadd", [T, FG], F32, isOutput=False)
    obn = nc.declare_dram_parameter("obn", [T, 2], F32, isOutput=True)
    ofg = nc.declare_dram_parameter("ofg", [T, FG], F32, isOutput=True)

    with tile.TileContext(nc) as tc:
        from contextlib import ExitStack

        with ExitStack() as ctx:
            const = ctx.enter_context(tc.tile_pool(name="const", bufs=1))
            xpool = ctx.enter_context(tc.tile_pool(name="xpool", bufs=1))
            w1pool = ctx.enter_context(tc.tile_pool(name="w1pool", bufs=1))
            w2pool = ctx.enter_context(tc.tile_pool(name="w2pool", bufs=1))
            hpool = ctx.enter_context(tc.tile_pool(name="hpool", bufs=1))
            fgopool = ctx.enter_context(tc.tile_pool(name="fgopool", bufs=1))
            smax = ctx.enter_context(tc.tile_pool(name="smax", bufs=1))
            psum_pool = ctx.enter_context(
                tc.tile_pool(name="psum_pool", bufs=8, space="PSUM")
            )

            # --- input DMAs -------------------------------------------------
            # x-path inputs on the ACT HWDGE ring so they don't queue behind
            # the big W1 chunks on the SP ring.
            roit_sb = const.tile([P, BPC * S], DT1, tag="roit")
            nc.scalar.dma_start(out=roit_sb[:], in_=roit[:])
            poolf_sb = const.tile([P, BPC * RNN], DT1, tag="poolf")
            nc.scalar.dma_start(out=poolf_sb[:], in_=poolf[:])
            xr_sb = const.tile([P, KT2 * T], DT1, tag="xr")
            nc.scalar.dma_start(out=xr_sb[:], in_=xr[:])

            w1_sb = []
            for q in range(W1_CHUNKS):
                t_ = w1pool.tile([P, KO_PER_CHUNK * 2 * RNN], DT1, tag=f"w1_{q}")
                nc.sync.dma_start(
                    out=t_[:],
                    in_=w1t[
                        :,
                        q * KO_PER_CHUNK * 2 * RNN : (q + 1) * KO_PER_CHUNK * 2 * RNN,
                    ],
                )
                w1_sb.append(t_)

            w2_sb = w2pool.tile([P, KT2 * FGP], DT2, tag="w2")
            nc.scalar.dma_start(out=w2_sb[:], in_=w2t[:])
            emb_sb = w2pool.tile([P, MT2 * FG], DT2, tag="emb")
            nc.scalar.dma_start(out=emb_sb[:], in_=embt[:])
            w2bn_sb = const.tile([P, KT2 * 2], DT2, tag="w2bn")
            nc.scalar.dma_start(out=w2bn_sb[:], in_=w2bnt[:])
            b1_sb = const.tile([P, MT1], F32, tag="b1")
            nc.scalar.dma_start(out=b1_sb[:], in_=b1[:])
            bfg2_sb = const.tile([P, MT2], F32, tag="bfg2")
            nc.scalar.dma_start(out=bfg2_sb[:], in_=bfg2[:])
            bnb_sb = const.tile([1, 2], DT2, tag="bnb")
            nc.scalar.dma_start(out=bnb_sb[:], in_=bnb[:])
            maskadd_sb = const.tile([T, FG], F32, tag="maskadd")
            nc.scalar.dma_start(out=maskadd_sb[:], in_=maskadd[:])

            ones_sb = const.tile([1, T], DT2, tag="ones")
            nc.vector.memset(ones_sb[:], 1.0)

            # --- pooled (feature-major) -------------------------------------
            xp_sb = xpool.tile([P, KT2 * T], DT1, tag="xp")
            for mf in range(KT2):
                psum = psum_pool.tile([P, T], F32, tag="ps", name=f"pp_{mf}")
                for b_ in range(BPC):
                    nc.tensor.matmul(
                        psum[:, b_ * S : (b_ + 1) * S],
                        lhsT=poolf_sb[:, b_ * RNN + mf * P : b_ * RNN + mf * P + P],
                        rhs=roit_sb[:, b_ * S : (b_ + 1) * S],
                        start=True,
                        stop=True,
                    )
                nc.vector.tensor_copy(xp_sb[:, mf * T : (mf + 1) * T], psum[:])

            def x_tile(k):
                if k < KT2:
                    return xr_sb[:, k * T : (k + 1) * T]
                return xp_sb[:, (k - KT2) * T : (k - KT2 + 1) * T]

            # --- MLP1: two halves, each K-outer with 8 bank accumulators ----
            h_sb = [None] * MT1
            for half in range(2):
                psums = [
                    psum_pool.tile([P, T], F32, tag="ps", name=f"ph_{half}_{i}")
                    for i in range(8)
                ]
                for k in range(KT1):
                    q, kl = divmod(k, KO_PER_CHUNK)
                    for i in range(8):
                        m = half * 8 + i
                        lhsT = w1_sb[q][
                            :, kl * 2 * RNN + m * P : kl * 2 * RNN + m * P + P
                        ]
                        nc.tensor.matmul(
                            psums[i][:],
                            lhsT=lhsT,
                            rhs=x_tile(k),
                            start=(k == 0),
                            stop=(k == KT1 - 1),
                        )
                for i in range(8):
                    m = half * 8 + i
                    h = hpool.tile([P, T], DT2, tag=f"h_{m}", name=f"h_{m}")
                    nc.vector.tensor_scalar(
                        h[:],
                        psums[i][:],
                        b1_sb[:, m : m + 1],
                        0.0,
                        op0=ALU.add,
                        op1=ALU.max,
                    )
                    h_sb[m] = h

            # --- bn head: logits (64, 2), then log_softmax ------------------
            psum_bn = psum_pool.tile([T, 2], F32, tag="ps", name="psum_bn")
            for k in range(KT2):
                nc.tensor.matmul(
                    psum_bn[:],
                    lhsT=h_sb[k][:],
                    rhs=w2bn_sb[:, k * 2 : (k + 1) * 2],
                    start=(k == 0),
                    stop=False,
                )
            nc.tensor.matmul(
                psum_bn[:], lhsT=ones_sb[:], rhs=bnb_sb[:], start=False, stop=True
            )

            neg_mx_bn = smax.tile([T, 1], F32, tag="neg_mx_bn")
            nc.vector.tensor_reduce(
                neg_mx_bn[:], psum_bn[:], axis=AX.X, op=ALU.max, negate=True
            )
            e_bn = smax.tile([T, 2], F32, tag="e_bn")
            s_bn = smax.tile([T, 1], F32, tag="s_bn")
            nc.scalar.activation(
                e_bn[:], psum_bn[:], AF.Exp, bias=neg_mx_bn[:], accum_out=s_bn[:]
            )
            ln_bn = smax.tile([T, 1], F32, tag="ln_bn")
            nc.scalar.activation(ln_bn[:], s_bn[:], AF.Ln)
            off_bn = smax.tile([T, 1], F32, tag="off_bn")
            nc.vector.tensor_sub(off_bn[:], neg_mx_bn[:], ln_bn[:])
            obn_st = smax.tile([T, 2], F32, tag="obn_st")
            nc.vector.tensor_scalar_add(obn_st[:], psum_bn[:], off_bn[:])
            nc.sync.dma_start(out=obn[:], in_=obn_st[:])

            # --- fg head: fg_outT (padded 384), score, log_softmax ----------
            fgo_sb = []
            for mt in range(MT2):
                psum = psum_pool.tile([P, T], F32, tag="ps", name=f"pfgo_{mt}")
                for k in range(KT2):
                    nc.tensor.matmul(
                        psum[:],
                        lhsT=w2_sb[:, k * FGP + mt * P : k * FGP + mt * P + P],
                        rhs=h_sb[KT2 + k][:],
                        start=(k == 0),
                        stop=(k == KT2 - 1),
                    )
                fgo = fgopool.tile([P, T], DT2, tag=f"fgo_{mt}")
                nc.vector.tensor_scalar_add(fgo[:], psum[:], bfg2_sb[:, mt : mt + 1])
                fgo_sb.append(fgo)

            psum_s = psum_pool.tile([T, FG], F32, tag="ps", name="psum_s")
            for kt in range(MT2):
                nc.tensor.matmul(
                    psum_s[:],
                    lhsT=fgo_sb[kt][:],
                    rhs=emb_sb[:, kt * FG : (kt + 1) * FG],
                    start=(kt == 0),
                    stop=(kt == MT2 - 1),
                )

            t2 = smax.tile([T, FG], F32, tag="t2")
            nc.vector.tensor_add(t2[:], psum_s[:], maskadd_sb[:])
            neg_mx = smax.tile([T, 1], F32, tag="neg_mx")
            nc.vector.tensor_reduce(
                neg_mx[:], t2[:], axis=AX.X, op=ALU.max, negate=True
            )
            e_fg = smax.tile([T, FG], F32, tag="e_fg")
            s_fg = smax.tile([T, 1], F32, tag="s_fg")
            nc.scalar.activation(
                e_fg[:], t2[:], AF.Exp, bias=neg_mx[:], accum_out=s_fg[:]
            )
            ln_fg = smax.tile([T, 1], F32, tag="ln_fg")
            nc.scalar.activation(ln_fg[:], s_fg[:], AF.Ln)
            off_fg = smax.tile([T, 1], F32, tag="off_fg")
            nc.vector.tensor_sub(off_fg[:], neg_mx[:], ln_fg[:])
            ofg_st = smax.tile([T, FG], F32, tag="ofg_st")
            nc.vector.tensor_scalar_add(ofg_st[:], t2[:], off_fg[:])
            nc.sync.dma_start(out=ofg[:], in_=ofg_st[:])

    nc.compile()
    _BUILD_CACHE[mode] = nc
    return nc


def _prep_inputs(inputs: dict, mode: str):
    np_dt1, np_dt2 = _np_dts(mode)

    fg_idx = np.asarray(inputs["fg_idx"])
    pool_feats = np.asarray(inputs["pool_feats"], dtype=np.float32)
    rnn_outs = np.asarray(inputs["rnn_outs"], dtype=np.float32)
    roi_labels = np.asarray(inputs["roi_labels"], dtype=np.float32)
    W_bn1 = np.asarray(inputs["W_bn1"], dtype=np.float32)
    b_bn1 = np.asarray(inputs["b_bn1"], dtype=np.float32)
    W_bn2 = np.asarray(inputs["W_bn2"], dtype=np.float32)
    b_bn2 = np.asarray(inputs["b_bn2"], dtype=np.float32)
    W_fg1 = np.asarray(inputs["W_fg1"], dtype=np.float32)
    b_fg1 = np.asarray(inputs["b_fg1"], dtype=np.float32)
    W_fg2 = np.asarray(inputs["W_fg2"], dtype=np.float32)
    b_fg2 = np.asarray(inputs["b_fg2"], dtype=np.float32)
    fg_emb = np.asarray(inputs["fg_emb"], dtype=np.float32)
    fg_mask = np.asarray(inputs["fg_mask"])

    # Shared (replicated) tensors.
    W1cat = np.concatenate([W_bn1, W_fg1], axis=0)          # (2048, 2048)
    w1t = _pack_kmajor(W1cat.T.copy()).astype(np_dt1)
    W2p = np.zeros((FGP, RNN), np.float32)
    W2p[:300] = W_fg2
    w2t = _pack_kmajor(np.ascontiguousarray(W2p.T)).astype(np_dt2)
    w2bnt = _pack_kmajor(np.ascontiguousarray(W_bn2.T)).astype(np_dt2)
    embp = np.zeros((FGP, FG), np.float32)
    embp[:300] = fg_emb.T
    embt = _pack_kmajor(embp).astype(np_dt2)
    b1 = np.concatenate([b_bn1, b_fg1]).reshape(MT1, P).T.copy().astype(np.float32)
    bfg2p = np.zeros((FGP,), np.float32)
    bfg2p[:300] = b_fg2
    bfg2 = bfg2p.reshape(MT2, P).T.copy().astype(np.float32)
    bnb = b_bn2.reshape(1, 2).astype(np_dt2)

    # Per-token additive mask.
    maskadd_full = fg_mask[fg_idx].astype(np.float32) * np.float32(MIN_VALUE)

    cnt = roi_labels.sum(axis=2)
    cnt = np.where(cnt == 0, 1.0, cnt).astype(np.float32)
    roi_scaled = roi_labels / cnt[..., None]                # (B, S, R)

    in_maps = []
    for c in range(N_CORES):
        b0 = c * BPC
        pf = pool_feats[b0 : b0 + BPC]                      # (4, 128, 1024)
        poolf_c = np.ascontiguousarray(
            pf.transpose(1, 0, 2).reshape(P, BPC * RNN)
        ).astype(np_dt1)
        rt = roi_scaled[b0 : b0 + BPC].transpose(0, 2, 1)   # (4, 128, 16)
        roit_c = np.ascontiguousarray(
            rt.transpose(1, 0, 2).reshape(P, BPC * S)
        ).astype(np_dt1)
        xrT = rnn_outs[b0 : b0 + BPC].reshape(T, RNN).T     # (1024, 64)
        xr_c = _pack_kmajor(np.ascontiguousarray(xrT)).astype(np_dt1)
        maskadd_c = np.ascontiguousarray(
            maskadd_full[b0 : b0 + BPC].reshape(T, FG)
        )
        in_maps.append(
            {
                "w1t": w1t,
                "w2t": w2t,
                "w2bnt": w2bnt,
                "embt": embt,
                "xr": xr_c,
                "poolf": poolf_c,
                "roit": roit_c,
                "b1": b1,
                "bfg2": bfg2,
                "bnb": bnb,
                "maskadd": maskadd_c,
            }
        )
    return in_maps


def kernel(**inputs):
    mode = COMPUTE_DTYPE
    nc = _build(mode)
    in_maps = _prep_inputs(inputs, mode)
    res = run_bass_kernel_spmd(nc, in_maps, list(range(N_CORES)))
    bn = np.stack(
        [res.results[c]["obn"].reshape(BPC, S, 2) for c in range(N_CORES)]
    ).reshape(B, S, 2)
    fg = np.stack(
        [res.results[c]["ofg"].reshape(BPC, S, FG) for c in range(N_CORES)]
    ).reshape(B, S, FG)
    return bn.astype(np.float32), fg.astype(np.float32)


# revision 12
# speedup vs baseline: 1.3073x; 1.3073x over previous
"""Trainium2 Bass kernel for nn_CascadeCore_3882650436478.

Data-parallel over batch: 8 NeuronCores x 4 batches (64 tokens each).
Weights are replicated; no cross-device communication.

Per-core dataflow (all matmul contractions on the partition dim, no
on-chip transposes — host pre-packs everything K-major):

  pooledT[f,t] = pool_feats[b].T-layout @ roiT_scaled[b]   (cnt folded in)
  xT = [rnnT; pooledT]                    (16 K-tiles of (128, 64))
  hT[m] = relu(W1catT[:,m].T @ xT + b1)   (16 M-tiles: 8 fg hid + 8 bn hid)
  fg_outT[mt] = W_fg2T[:,mt].T @ h_fgT + b_fg2         (3 M-tiles, padded 300->384)
  score = fg_outT.T @ fg_embT + maskadd                (64, 461)
  bn_logits = h_bnT.T @ W_bn2.T + ones.T @ b_bn2       (64, 2)
  out = log_softmax over free dim for both branches

W1 streams in four M-major chunks (each chunk = all 16 K-tiles for 4
M-tiles), so each chunk's h tiles complete as soon as that chunk lands
and the second-stage matmuls accumulate under the remaining DMA
stream. The fg half (long tail: fg2 -> score -> softmax) is processed
first so its tail overlaps W1's bn chunks; only the short bn tail runs
after the last DMA. ACT warmup activations pull the LUT load off the
critical path.

Compute dtype modes (BASS_CASCADE_DTYPE): "bf16" (both stages bf16),
"mixed" (stage-1 bf16, stage-2 f32), "f32".
"""

import os
from contextlib import ExitStack

import numpy as np
import ml_dtypes

import concourse.bass as bass
import concourse.mybir as mybir
import concourse.tile as tile
from concourse import bacc
from concourse.bass_utils import run_bass_kernel_spmd


def _patch_act_tables():
    """Steer bacc's greedy ACT-table-set chooser to the combined
    Exp+Ln set so the kernel needs exactly one LUT load. Set ids are
    positional, so contents are masked (never reordered): sets other
    than natural_log_exp_and_others stop advertising Exp/Ln.
    """
    import concourse.bacc as _bacc_mod
    from concourse.hw_specs import get_activation_tables as _orig

    def patched(module_arch):
        tabs = _orig(module_arch)
        exp = mybir.ActivationFunctionType.Exp
        ln = mybir.ActivationFunctionType.Ln
        if any(exp in s and ln in s for s in tabs.values()):
            for name, s in tabs.items():
                if not (exp in s and ln in s):
                    s.discard(exp)
                    s.discard(ln)
        return tabs

    _bacc_mod.get_activation_tables = patched


_patch_act_tables()

B, S, R = 32, 16, 128
RNN = 1024
FG = 461
MIN_VALUE = -1e8

N_CORES = 8
BPC = B // N_CORES          # batches per core
T = BPC * S                 # tokens per core (64)
P = 128
KT1 = (2 * RNN) // P        # 16 K-tiles for MLP1
MT1 = (2 * RNN) // P        # 16 M-tiles (8 fg hid then 8 bn hid)
KT2 = RNN // P              # 8 K-tiles for MLP2
FGP = 384                   # fg_out features padded 300 -> 384
MT2 = FGP // P              # 3 M-tiles for fg_out
# M-major W1 chunks; tapered so the post-DMA bn tail is short.
CHUNK_MS = [[0, 1, 2, 3], [4, 5, 6, 7], [8, 9, 10, 11], [12, 13], [14], [15]]
W1_CHUNKS = len(CHUNK_MS)
CHUNK_OFF = [0]
for _ms in CHUNK_MS:
    CHUNK_OFF.append(CHUNK_OFF[-1] + KT1 * len(_ms) * P)
W1_FREE = CHUNK_OFF[-1]                 # total free elements per partition

COMPUTE_DTYPE = os.environ.get("BASS_CASCADE_DTYPE", "mixed")

_BUILD_CACHE = {}


def _dts(mode):
    bf, f32 = mybir.dt.bfloat16, mybir.dt.float32
    return {"f32": (f32, f32), "mixed": (bf, f32), "bf16": (bf, bf)}[mode]


def _np_dts(mode):
    bf, f32 = ml_dtypes.bfloat16, np.float32
    return {"f32": (f32, f32), "mixed": (bf, f32), "bf16": (bf, bf)}[mode]


def _pack_kmajor(a: np.ndarray) -> np.ndarray:
    """(K, M) -> (128, K//128 * M); element (p, ko*M+m) = a[ko*128+p, m]."""
    K, M = a.shape
    assert K % P == 0
    return np.ascontiguousarray(
        a.reshape(K // P, P, M).transpose(1, 0, 2).reshape(P, (K // P) * M)
    )


def _build(mode: str):
    if mode in _BUILD_CACHE:
        return _BUILD_CACHE[mode]

    DT1, DT2 = _dts(mode)
    F32 = mybir.dt.float32
    AF = mybir.ActivationFunctionType
    ALU = mybir.AluOpType
    AX = mybir.AxisListType

    nc = bacc.Bacc(
        "TRN2", target_bir_lowering=False, debug=False, enable_asserts=False
    )

    XIN_COLS = BPC * RNN + KT2 * T + BPC * S      # poolf | xr | roit
    w1t = nc.declare_dram_parameter("w1t", [P, W1_FREE], DT1, isOutput=False)
    xin = nc.declare_dram_parameter("xin", [P, XIN_COLS], DT1, isOutput=False)
    w2t = nc.declare_dram_parameter("w2t", [P, KT2 * FGP], DT2, isOutput=False)
    embt = nc.declare_dram_parameter("embt", [P, MT2 * FG], DT2, isOutput=False)
    sm2 = nc.declare_dram_parameter("sm2", [P, KT2 * 2 + 2], DT2, isOutput=False)
    smf = nc.declare_dram_parameter("smf", [P, MT1 + MT2], F32, isOutput=False)
    maskadd = nc.declare_dram_parameter("maskadd", [T, FG], F32, isOutput=False)
    oall = nc.declare_dram_parameter("oall", [T, FG + 2], F32, isOutput=True)

    with tile.TileContext(nc) as tc:
        with ExitStack() as ctx:
            const = ctx.enter_context(tc.tile_pool(name="const", bufs=1))
            xpool = ctx.enter_context(tc.tile_pool(name="xpool", bufs=1))
            w1pool = ctx.enter_context(tc.tile_pool(name="w1pool", bufs=1))
            w2pool = ctx.enter_context(tc.tile_pool(name="w2pool", bufs=1))
            hpool = ctx.enter_context(tc.tile_pool(name="hpool", bufs=1))
            fgopool = ctx.enter_context(tc.tile_pool(name="fgopool", bufs=1))
            smax = ctx.enter_context(tc.tile_pool(name="smax", bufs=1))
            psum_pool = ctx.enter_context(
                tc.tile_pool(name="psum_pool", bufs=8, space="PSUM")
            )

            # --- ACT warmup: pull the LUT-set load off the critical path ----
            warm = const.tile([T, 1], F32, tag="warm")
            nc.vector.memset(warm[:], 1.0)
            warm_o = const.tile([T, 1], F32, tag="warm_o")
            nc.scalar.activation(warm_o[:], warm[:], AF.Ln)
            nc.scalar.activation(warm_o[:], warm[:], AF.Exp)

            # --- input DMAs, in need-order on the SP HWDGE ring -------------
            xin_sb = const.tile([P, XIN_COLS], DT1, tag="xin")
            nc.sync.dma_start(out=xin_sb[:], in_=xin[:])
            poolf_sb = xin_sb[:, 0 : BPC * RNN]
            xr_sb = xin_sb[:, BPC * RNN : BPC * RNN + KT2 * T]
            roit_sb = xin_sb[:, BPC * RNN + KT2 * T :]
            smf_sb = const.tile([P, MT1 + MT2], F32, tag="smf")
            nc.sync.dma_start(out=smf_sb[:], in_=smf[:])
            b1_sb = smf_sb[:, 0:MT1]
            bfg2_sb = smf_sb[:, MT1:]
            w1_sb = []
            for q in range(W1_CHUNKS):
                t_ = w1pool.tile(
                    [P, CHUNK_OFF[q + 1] - CHUNK_OFF[q]], DT1,
                    tag=f"w1_{q}", name=f"w1c{q}",
                )
                w1_sb.append(t_)

            def dma_chunk(q):
                nc.sync.dma_start(
                    out=w1_sb[q][:], in_=w1t[:, CHUNK_OFF[q] : CHUNK_OFF[q + 1]]
                )

            dma_chunk(0)
            w2_sb = w2pool.tile([P, KT2 * FGP], DT2, tag="w2")
            nc.sync.dma_start(out=w2_sb[:], in_=w2t[:])
            dma_chunk(1)
            emb_sb = w2pool.tile([P, MT2 * FG], DT2, tag="emb")
            nc.sync.dma_start(out=emb_sb[:], in_=embt[:])
            maskadd_sb = const.tile([T, FG], F32, tag="maskadd")
            nc.sync.dma_start(out=maskadd_sb[:], in_=maskadd[:])
            sm2_sb = const.tile([P, KT2 * 2 + 2], DT2, tag="sm2")
            nc.sync.dma_start(out=sm2_sb[:], in_=sm2[:])
            w2bn_sb = sm2_sb[:, 0 : KT2 * 2]
            bnb_sb = sm2_sb[0:1, KT2 * 2 : KT2 * 2 + 2]
            for q in range(2, W1_CHUNKS):
                dma_chunk(q)

            ones_sb = const.tile([1, T], DT2, tag="ones")
            nc.vector.memset(ones_sb[:], 1.0)

            # --- pooled (feature-major) -------------------------------------
            xp_sb = xpool.tile([P, KT2 * T], DT1, tag="xp")
            for mf in range(KT2):
                psum = psum_pool.tile([P, T], F32, tag="ps", name=f"pp_{mf}")
                for b_ in range(BPC):
                    nc.tensor.matmul(
                        psum[:, b_ * S : (b_ + 1) * S],
                        lhsT=poolf_sb[:, b_ * RNN + mf * P : b_ * RNN + mf * P + P],
                        rhs=roit_sb[:, b_ * S : (b_ + 1) * S],
                        start=True,
                        stop=True,
                    )
                nc.vector.tensor_copy(xp_sb[:, mf * T : (mf + 1) * T], psum[:])

            def x_tile(k):
                if k < KT2:
                    return xr_sb[:, k * T : (k + 1) * T]
                return xp_sb[:, (k - KT2) * T : (k - KT2 + 1) * T]

            # --- MLP1 (M-major chunks) + fused second-stage accumulation ----
            # m 0..7  = fg hidden (chunks 0-1): feed fg2 accumulation
            # m 8..15 = bn hidden (chunks 2-3): feed bn2 accumulation
            psum_fgo = [
                psum_pool.tile([P, T], F32, tag="ps", name=f"pfgo_{mt}")
                for mt in range(MT2)
            ]
            psum_bn = psum_pool.tile([T, 2], F32, tag="ps", name="psum_bn")
            h_sb = [None] * MT1
            fgo_sb = []
            for q in range(W1_CHUNKS):
                chunk_cols = len(CHUNK_MS[q]) * P
                for ml, m in enumerate(CHUNK_MS[q]):
                    psum = psum_pool.tile([P, T], F32, tag="ps", name=f"ph_{m}")
                    for k in range(KT1):
                        nc.tensor.matmul(
                            psum[:],
                            lhsT=w1_sb[q][
                                :,
                                k * chunk_cols + ml * P : k * chunk_cols + ml * P + P,
                            ],
                            rhs=x_tile(k),
                            start=(k == 0),
                            stop=(k == KT1 - 1),
                        )
                    h = hpool.tile([P, T], DT2, tag=f"h_{m}", name=f"h_{m}")
                    nc.vector.tensor_scalar(
                        h[:],
                        psum[:],
                        b1_sb[:, m : m + 1],
                        0.0,
                        op0=ALU.add,
                        op1=ALU.max,
                    )
                    h_sb[m] = h

                    if m < KT2:
                        # fg half: accumulate fg2 with this h as K-tile m
                        for mt in range(MT2):
                            nc.tensor.matmul(
                                psum_fgo[mt][:],
                                lhsT=w2_sb[:, m * FGP + mt * P : m * FGP + mt * P + P],
                                rhs=h[:],
                                start=(m == 0),
                                stop=(m == KT2 - 1),
                            )
                    else:
                        i = m - KT2
                        nc.tensor.matmul(
                            psum_bn[:],
                            lhsT=h[:],
                            rhs=w2bn_sb[:, i * 2 : (i + 1) * 2],
                            start=(i == 0),
                            stop=False,
                        )

                if CHUNK_MS[q][-1] == KT2 - 1:
                    # fg half complete: fgo eviction, score, fg log_softmax
                    for mt in range(MT2):
                        fgo = fgopool.tile(
                            [P, T], DT2, tag=f"fgo_{mt}", name=f"fgo_{mt}"
                        )
                        nc.vector.tensor_scalar_add(
                            fgo[:], psum_fgo[mt][:], bfg2_sb[:, mt : mt + 1]
                        )
                        fgo_sb.append(fgo)
                    psum_s = psum_pool.tile([T, FG], F32, tag="ps", name="psum_s")
                    for kt in range(MT2):
                        nc.tensor.matmul(
                            psum_s[:],
                            lhsT=fgo_sb[kt][:],
                            rhs=emb_sb[:, kt * FG : (kt + 1) * FG],
                            start=(kt == 0),
                            stop=(kt == MT2 - 1),
                        )
                    t2 = smax.tile([T, FG], F32, tag="t2")
                    nc.vector.tensor_add(t2[:], psum_s[:], maskadd_sb[:])
                    neg_mx = smax.tile([T, 1], F32, tag="neg_mx")
                    nc.vector.tensor_reduce(
                        neg_mx[:], t2[:], axis=AX.X, op=ALU.max, negate=True
                    )
                    e_fg = smax.tile([T, FG], F32, tag="e_fg")
                    s_fg = smax.tile([T, 1], F32, tag="s_fg")
                    nc.scalar.activation(
                        e_fg[:], t2[:], AF.Exp, bias=neg_mx[:], accum_out=s_fg[:]
                    )
                    ln_fg = smax.tile([T, 1], F32, tag="ln_fg")
                    nc.scalar.activation(ln_fg[:], s_fg[:], AF.Ln)
                    off_fg = smax.tile([T, 1], F32, tag="off_fg")
                    nc.vector.tensor_sub(off_fg[:], neg_mx[:], ln_fg[:])
                    out_st = smax.tile([T, FG + 2], F32, tag="out_st")
                    nc.vector.tensor_scalar_add(
                        out_st[:, 0:FG], t2[:], off_fg[:]
                    )

            # bn bias via ones-row matmul, then bn log_softmax
            nc.tensor.matmul(
                psum_bn[:], lhsT=ones_sb[:], rhs=bnb_sb[:], start=False, stop=True
            )
            # bn logits are O(1): exp is safe without max subtraction,
            # keeping the post-DMA tail to Exp -> Ln -> sub.
            e_bn = smax.tile([T, 2], F32, tag="e_bn")
            s_bn = smax.tile([T, 1], F32, tag="s_bn")
            nc.scalar.activation(
                e_bn[:], psum_bn[:], AF.Exp, accum_out=s_bn[:]
            )
            ln_bn = smax.tile([T, 1], F32, tag="ln_bn")
            nc.scalar.activation(ln_bn[:], s_bn[:], AF.Ln)
            nc.vector.tensor_scalar(
                out_st[:, FG : FG + 2], psum_bn[:], ln_bn[:], None, op0=ALU.subtract
            )
            nc.sync.dma_start(out=oall[:], in_=out_st[:])

    nc.compile()
    _BUILD_CACHE[mode] = nc
    return nc


def _prep_inputs(inputs: dict, mode: str):
    np_dt1, np_dt2 = _np_dts(mode)

    fg_idx = np.asarray(inputs["fg_idx"])
    pool_feats = np.asarray(inputs["pool_feats"], dtype=np.float32)
    rnn_outs = np.asarray(inputs["rnn_outs"], dtype=np.float32)
    roi_labels = np.asarray(inputs["roi_labels"], dtype=np.float32)
    W_bn1 = np.asarray(inputs["W_bn1"], dtype=np.float32)
    b_bn1 = np.asarray(inputs["b_bn1"], dtype=np.float32)
    W_bn2 = np.asarray(inputs["W_bn2"], dtype=np.float32)
    b_bn2 = np.asarray(inputs["b_bn2"], dtype=np.float32)
    W_fg1 = np.asarray(inputs["W_fg1"], dtype=np.float32)
    b_fg1 = np.asarray(inputs["b_fg1"], dtype=np.float32)
    W_fg2 = np.asarray(inputs["W_fg2"], dtype=np.float32)
    b_fg2 = np.asarray(inputs["b_fg2"], dtype=np.float32)
    fg_emb = np.asarray(inputs["fg_emb"], dtype=np.float32)
    fg_mask = np.asarray(inputs["fg_mask"])

    # Shared (replicated) tensors. M order: fg hidden first, then bn.
    W1cat = np.concatenate([W_fg1, W_bn1], axis=0)          # (2048, 2048)
    W1T = np.ascontiguousarray(W1cat.T)                     # (2048 in, 2048 out)
    w1t = np.concatenate(
        [
            _pack_kmajor(
                np.ascontiguousarray(W1T[:, [mm * P + j for mm in ms for j in range(P)]])
            )
            for ms in CHUNK_MS
        ],
        axis=1,
    ).astype(np_dt1)
    W2p = np.zeros((FGP, RNN), np.float32)
    W2p[:300] = W_fg2
    w2t = _pack_kmajor(np.ascontiguousarray(W2p.T)).astype(np_dt2)
    w2bnt = _pack_kmajor(np.ascontiguousarray(W_bn2.T))
    sm2 = np.zeros((P, KT2 * 2 + 2), np.float32)
    sm2[:, 0 : KT2 * 2] = w2bnt
    sm2[0, KT2 * 2 :] = b_bn2
    sm2 = sm2.astype(np_dt2)
    embp = np.zeros((FGP, FG), np.float32)
    embp[:300] = fg_emb.T
    embt = _pack_kmajor(embp).astype(np_dt2)
    b1 = np.concatenate([b_fg1, b_bn1]).reshape(MT1, P).T
    bfg2p = np.zeros((FGP,), np.float32)
    bfg2p[:300] = b_fg2
    bfg2 = bfg2p.reshape(MT2, P).T
    smf = np.concatenate([b1, bfg2], axis=1).astype(np.float32)
    smf = np.ascontiguousarray(smf)

    # Per-token additive mask.
    maskadd_full = fg_mask[fg_idx].astype(np.float32) * np.float32(MIN_VALUE)

    cnt = roi_labels.sum(axis=2)
    cnt = np.where(cnt == 0, 1.0, cnt).astype(np.float32)
    roi_scaled = roi_labels / cnt[..., None]                # (B, S, R)

    in_maps = []
    for c in range(N_CORES):
        b0 = c * BPC
        pf = pool_feats[b0 : b0 + BPC]                      # (4, 128, 1024)
        poolf_c = np.ascontiguousarray(
            pf.transpose(1, 0, 2).reshape(P, BPC * RNN)
        ).astype(np_dt1)
        rt = roi_scaled[b0 : b0 + BPC].transpose(0, 2, 1)   # (4, 128, 16)
        roit_c = np.ascontiguousarray(
            rt.transpose(1, 0, 2).reshape(P, BPC * S)
        ).astype(np_dt1)
        xrT = rnn_outs[b0 : b0 + BPC].reshape(T, RNN).T     # (1024, 64)
        xr_c = _pack_kmajor(np.ascontiguousarray(xrT)).astype(np_dt1)
        xin_c = np.ascontiguousarray(
            np.concatenate([poolf_c, xr_c, roit_c], axis=1)
        )
        maskadd_c = np.ascontiguousarray(
            maskadd_full[b0 : b0 + BPC].reshape(T, FG)
        )
        in_maps.append(
            {
                "w1t": w1t,
                "w2t": w2t,
                "embt": embt,
                "xin": xin_c,
                "sm2": sm2,
                "smf": smf,
                "maskadd": maskadd_c,
            }
        )
    return in_maps


def kernel(**inputs):
    mode = COMPUTE_DTYPE
    nc = _build(mode)
    in_maps = _prep_inputs(inputs, mode)
    res = run_bass_kernel_spmd(nc, in_maps, list(range(N_CORES)))
    oall = np.stack([res.results[c]["oall"] for c in range(N_CORES)])
    bn = oall[:, :, FG:].reshape(B, S, 2)
    fg = oall[:, :, :FG].reshape(B, S, FG)
    return (
        np.ascontiguousarray(bn).astype(np.float32),
        np.ascontiguousarray(fg).astype(np.float32),
    )


# revision 14
# speedup vs baseline: 1.3106x; 1.0025x over previous
"""Trainium2 Bass kernel for nn_CascadeCore_3882650436478.

Data-parallel over batch: 8 NeuronCores x 4 batches (64 tokens each).
Weights are replicated; no cross-device communication.

Per-core dataflow (all matmul contractions on the partition dim, no
on-chip transposes — host pre-packs everything K-major):

  pooledT[f,t] = pool_feats[b].T-layout @ roiT_scaled[b]   (cnt folded in)
  xT = [rnnT; pooledT]                    (16 K-tiles of (128, 64))
  hT[m] = relu(W1catT[:,m].T @ xT + b1)   (16 M-tiles: 8 fg hid + 8 bn hid)
  fg_outT[mt] = W_fg2T[:,mt].T @ h_fgT + b_fg2         (3 M-tiles, padded 300->384)
  score = fg_outT.T @ fg_embT + maskadd                (64, 461)
  bn_logits = h_bnT.T @ W_bn2.T + ones.T @ b_bn2       (64, 2)
  out = log_softmax over free dim for both branches

W1 streams in four M-major chunks (each chunk = all 16 K-tiles for 4
M-tiles), so each chunk's h tiles complete as soon as that chunk lands
and the second-stage matmuls accumulate under the remaining DMA
stream. The fg half (long tail: fg2 -> score -> softmax) is processed
first so its tail overlaps W1's bn chunks; only the short bn tail runs
after the last DMA. ACT warmup activations pull the LUT load off the
critical path.

Compute dtype modes (BASS_CASCADE_DTYPE): "bf16" (both stages bf16),
"mixed" (stage-1 bf16, stage-2 f32), "f32".
"""

import os
from contextlib import ExitStack

import numpy as np
import ml_dtypes

import concourse.bass as bass
import concourse.mybir as mybir
import concourse.tile as tile
from concourse import bacc
from concourse.bass_utils import run_bass_kernel_spmd


def _patch_act_tables():
    """Steer bacc's greedy ACT-table-set chooser to the combined
    Exp+Ln set so the kernel needs exactly one LUT load. Set ids are
    positional, so contents are masked (never reordered): sets other
    than natural_log_exp_and_others stop advertising Exp/Ln.
    """
    import concourse.bacc as _bacc_mod
    from concourse.hw_specs import get_activation_tables as _orig

    def patched(module_arch):
        tabs = _orig(module_arch)
        exp = mybir.ActivationFunctionType.Exp
        ln = mybir.ActivationFunctionType.Ln
        if any(exp in s and ln in s for s in tabs.values()):
            for name, s in tabs.items():
                if not (exp in s and ln in s):
                    s.discard(exp)
                    s.discard(ln)
        return tabs

    _bacc_mod.get_activation_tables = patched


_patch_act_tables()

B, S, R = 32, 16, 128
RNN = 1024
FG = 461
MIN_VALUE = -1e8

N_CORES = 8
BPC = B // N_CORES          # batches per core
T = BPC * S                 # tokens per core (64)
P = 128
KT1 = (2 * RNN) // P        # 16 K-tiles for MLP1
MT1 = (2 * RNN) // P        # 16 M-tiles (8 fg hid then 8 bn hid)
KT2 = RNN // P              # 8 K-tiles for MLP2
FGP = 384                   # fg_out features padded 300 -> 384
MT2 = FGP // P              # 3 M-tiles for fg_out
# M-major W1 chunks; tapered so the post-DMA bn tail is short.
CHUNK_MS = [[0, 1, 2, 3], [4, 5, 6, 7], [8, 9, 10, 11], [12, 13], [14], [15]]
W1_CHUNKS = len(CHUNK_MS)
CHUNK_OFF = [0]
for _ms in CHUNK_MS:
    CHUNK_OFF.append(CHUNK_OFF[-1] + KT1 * len(_ms) * P)
W1_FREE = CHUNK_OFF[-1]                 # total free elements per partition

COMPUTE_DTYPE = os.environ.get("BASS_CASCADE_DTYPE", "mixed")

_BUILD_CACHE = {}


def _dts(mode):
    bf, f32 = mybir.dt.bfloat16, mybir.dt.float32
    return {"f32": (f32, f32), "mixed": (bf, f32), "bf16": (bf, bf)}[mode]


def _np_dts(mode):
    bf, f32 = ml_dtypes.bfloat16, np.float32
    return {"f32": (f32, f32), "mixed": (bf, f32), "bf16": (bf, bf)}[mode]


def _pack_kmajor(a: np.ndarray) -> np.ndarray:
    """(K, M) -> (128, K//128 * M); element (p, ko*M+m) = a[ko*128+p, m]."""
    K, M = a.shape
    assert K % P == 0
    return np.ascontiguousarray(
        a.reshape(K // P, P, M).transpose(1, 0, 2).reshape(P, (K // P) * M)
    )


def _build(mode: str):
    if mode in _BUILD_CACHE:
        return _BUILD_CACHE[mode]

    DT1, DT2 = _dts(mode)
    F32 = mybir.dt.float32
    AF = mybir.ActivationFunctionType
    ALU = mybir.AluOpType
    AX = mybir.AxisListType

    nc = bacc.Bacc(
        "TRN2", target_bir_lowering=False, debug=False, enable_asserts=False
    )

    XIN_COLS = BPC * RNN + KT2 * T + BPC * S      # poolf | xr | roit
    w1t = nc.declare_dram_parameter("w1t", [P, W1_FREE], DT1, isOutput=False)
    xin = nc.declare_dram_parameter("xin", [P, XIN_COLS], DT1, isOutput=False)
    w2t = nc.declare_dram_parameter("w2t", [P, KT2 * FGP], DT2, isOutput=False)
    embt = nc.declare_dram_parameter("embt", [P, MT2 * FG], DT2, isOutput=False)
    sm2 = nc.declare_dram_parameter("sm2", [P, KT2 * 2 + 2], DT2, isOutput=False)
    smf = nc.declare_dram_parameter("smf", [P, MT1 + MT2], F32, isOutput=False)
    maskadd = nc.declare_dram_parameter("maskadd", [T, FG], F32, isOutput=False)
    obn = nc.declare_dram_parameter("obn", [T, 2], F32, isOutput=True)
    ofg = nc.declare_dram_parameter("ofg", [T, FG], F32, isOutput=True)

    with tile.TileContext(nc) as tc:
        with ExitStack() as ctx:
            const = ctx.enter_context(tc.tile_pool(name="const", bufs=1))
            xpool = ctx.enter_context(tc.tile_pool(name="xpool", bufs=1))
            w1pool = ctx.enter_context(tc.tile_pool(name="w1pool", bufs=1))
            w2pool = ctx.enter_context(tc.tile_pool(name="w2pool", bufs=1))
            hpool = ctx.enter_context(tc.tile_pool(name="hpool", bufs=1))
            fgopool = ctx.enter_context(tc.tile_pool(name="fgopool", bufs=1))
            smax = ctx.enter_context(tc.tile_pool(name="smax", bufs=1))
            psum_pool = ctx.enter_context(
                tc.tile_pool(name="psum_pool", bufs=8, space="PSUM")
            )

            # --- ACT warmup: pull the LUT-set load off the critical path ----
            warm = const.tile([T, 1], F32, tag="warm")
            nc.vector.memset(warm[:], 1.0)
            warm_o = const.tile([T, 1], F32, tag="warm_o")
            nc.scalar.activation(warm_o[:], warm[:], AF.Ln)
            nc.scalar.activation(warm_o[:], warm[:], AF.Exp)

            # --- input DMAs, in need-order on the SP HWDGE ring -------------
            xin_sb = const.tile([P, XIN_COLS], DT1, tag="xin")
            nc.sync.dma_start(out=xin_sb[:], in_=xin[:])
            poolf_sb = xin_sb[:, 0 : BPC * RNN]
            xr_sb = xin_sb[:, BPC * RNN : BPC * RNN + KT2 * T]
            roit_sb = xin_sb[:, BPC * RNN + KT2 * T :]
            smf_sb = const.tile([P, MT1 + MT2], F32, tag="smf")
            nc.sync.dma_start(out=smf_sb[:], in_=smf[:])
            b1_sb = smf_sb[:, 0:MT1]
            bfg2_sb = smf_sb[:, MT1:]
            w1_sb = []
            for q in range(W1_CHUNKS):
                t_ = w1pool.tile(
                    [P, CHUNK_OFF[q + 1] - CHUNK_OFF[q]], DT1,
                    tag=f"w1_{q}", name=f"w1c{q}",
                )
                w1_sb.append(t_)

            def dma_chunk(q):
                lo, hi = CHUNK_OFF[q], CHUNK_OFF[q + 1]
                if q == W1_CHUNKS - 1:
                    # split the final chunk so its first-half matmuls
                    # overlap the second half's DMA
                    mid_cols = (hi - lo) // 2
                    nc.sync.dma_start(
                        out=w1_sb[q][:, :mid_cols], in_=w1t[:, lo : lo + mid_cols]
                    )
                    nc.sync.dma_start(
                        out=w1_sb[q][:, mid_cols:], in_=w1t[:, lo + mid_cols : hi]
                    )
                else:
                    nc.sync.dma_start(out=w1_sb[q][:], in_=w1t[:, lo:hi])

            dma_chunk(0)
            w2_sb = w2pool.tile([P, KT2 * FGP], DT2, tag="w2")
            nc.sync.dma_start(out=w2_sb[:], in_=w2t[:])
            dma_chunk(1)
            emb_sb = w2pool.tile([P, MT2 * FG], DT2, tag="emb")
            nc.sync.dma_start(out=emb_sb[:], in_=embt[:])
            maskadd_sb = const.tile([T, FG], F32, tag="maskadd")
            nc.sync.dma_start(out=maskadd_sb[:], in_=maskadd[:])
            sm2_sb = const.tile([P, KT2 * 2 + 2], DT2, tag="sm2")
            nc.sync.dma_start(out=sm2_sb[:], in_=sm2[:])
            w2bn_sb = sm2_sb[:, 0 : KT2 * 2]
            bnb_sb = sm2_sb[0:1, KT2 * 2 : KT2 * 2 + 2]
            for q in range(2, W1_CHUNKS):
                dma_chunk(q)

            ones_sb = const.tile([1, T], DT2, tag="ones")
            nc.vector.memset(ones_sb[:], 1.0)

            # --- pooled (feature-major) -------------------------------------
            xp_sb = xpool.tile([P, KT2 * T], DT1, tag="xp")
            for mf in range(KT2):
                psum = psum_pool.tile([P, T], F32, tag="ps", name=f"pp_{mf}")
                for b_ in range(BPC):
                    nc.tensor.matmul(
                        psum[:, b_ * S : (b_ + 1) * S],
                        lhsT=poolf_sb[:, b_ * RNN + mf * P : b_ * RNN + mf * P + P],
                        rhs=roit_sb[:, b_ * S : (b_ + 1) * S],
                        start=True,
                        stop=True,
                    )
                nc.vector.tensor_copy(xp_sb[:, mf * T : (mf + 1) * T], psum[:])

            def x_tile(k):
                if k < KT2:
                    return xr_sb[:, k * T : (k + 1) * T]
                return xp_sb[:, (k - KT2) * T : (k - KT2 + 1) * T]

            # --- MLP1 (M-major chunks) + fused second-stage accumulation ----
            # m 0..7  = fg hidden (chunks 0-1): feed fg2 accumulation
            # m 8..15 = bn hidden (chunks 2-3): feed bn2 accumulation
            psum_fgo = [
                psum_pool.tile([P, T], F32, tag="ps", name=f"pfgo_{mt}")
                for mt in range(MT2)
            ]
            psum_bn = psum_pool.tile([T, 2], F32, tag="ps", name="psum_bn")
            h_sb = [None] * MT1
            fgo_sb = []
            for q in range(W1_CHUNKS):
                chunk_cols = len(CHUNK_MS[q]) * P
                for ml, m in enumerate(CHUNK_MS[q]):
                    psum = psum_pool.tile([P, T], F32, tag="ps", name=f"ph_{m}")
                    for k in range(KT1):
                        nc.tensor.matmul(
                            psum[:],
                            lhsT=w1_sb[q][
                                :,
                                k * chunk_cols + ml * P : k * chunk_cols + ml * P + P,
                            ],
                            rhs=x_tile(k),
                            start=(k == 0),
                            stop=(k == KT1 - 1),
                        )
                    h = hpool.tile([P, T], DT2, tag=f"h_{m}", name=f"h_{m}")
                    nc.vector.tensor_scalar(
                        h[:],
                        psum[:],
                        b1_sb[:, m : m + 1],
                        0.0,
                        op0=ALU.add,
                        op1=ALU.max,
                    )
                    h_sb[m] = h

                    if m < KT2:
                        # fg half: accumulate fg2 with this h as K-tile m
                        for mt in range(MT2):
                            nc.tensor.matmul(
                                psum_fgo[mt][:],
                                lhsT=w2_sb[:, m * FGP + mt * P : m * FGP + mt * P + P],
                                rhs=h[:],
                                start=(m == 0),
                                stop=(m == KT2 - 1),
                            )
                    else:
                        i = m - KT2
                        nc.tensor.matmul(
                            psum_bn[:],
                            lhsT=h[:],
                            rhs=w2bn_sb[:, i * 2 : (i + 1) * 2],
                            start=(i == 0),
                            stop=False,
                        )

                if CHUNK_MS[q][-1] == KT2 - 1:
                    # fg half complete: fgo eviction, score, fg log_softmax
                    for mt in range(MT2):
                        fgo = fgopool.tile(
                            [P, T], DT2, tag=f"fgo_{mt}", name=f"fgo_{mt}"
                        )
                        nc.vector.tensor_scalar_add(
                            fgo[:], psum_fgo[mt][:], bfg2_sb[:, mt : mt + 1]
                        )
                        fgo_sb.append(fgo)
                    psum_s = psum_pool.tile([T, FG], F32, tag="ps", name="psum_s")
                    for kt in range(MT2):
                        nc.tensor.matmul(
                            psum_s[:],
                            lhsT=fgo_sb[kt][:],
                            rhs=emb_sb[:, kt * FG : (kt + 1) * FG],
                            start=(kt == 0),
                            stop=(kt == MT2 - 1),
                        )
                    t2 = smax.tile([T, FG], F32, tag="t2")
                    nc.vector.tensor_add(t2[:], psum_s[:], maskadd_sb[:])
                    neg_mx = smax.tile([T, 1], F32, tag="neg_mx")
                    nc.vector.tensor_reduce(
                        neg_mx[:], t2[:], axis=AX.X, op=ALU.max, negate=True
                    )
                    e_fg = smax.tile([T, FG], F32, tag="e_fg")
                    s_fg = smax.tile([T, 1], F32, tag="s_fg")
                    nc.scalar.activation(
                        e_fg[:], t2[:], AF.Exp, bias=neg_mx[:], accum_out=s_fg[:]
                    )
                    ln_fg = smax.tile([T, 1], F32, tag="ln_fg")
                    nc.scalar.activation(ln_fg[:], s_fg[:], AF.Ln)
                    off_fg = smax.tile([T, 1], F32, tag="off_fg")
                    nc.vector.tensor_sub(off_fg[:], neg_mx[:], ln_fg[:])
                    ofg_st = smax.tile([T, FG], F32, tag="ofg_st")
                    nc.vector.tensor_scalar_add(ofg_st[:], t2[:], off_fg[:])
                    nc.sync.dma_start(out=ofg[:], in_=ofg_st[:])

            # bn bias via ones-row matmul, then bn log_softmax
            nc.tensor.matmul(
                psum_bn[:], lhsT=ones_sb[:], rhs=bnb_sb[:], start=False, stop=True
            )
            # bn logits are O(1): exp is safe without max subtraction,
            # keeping the post-DMA tail to Exp -> Ln -> sub.
            e_bn = smax.tile([T, 2], F32, tag="e_bn")
            s_bn = smax.tile([T, 1], F32, tag="s_bn")
            nc.scalar.activation(
                e_bn[:], psum_bn[:], AF.Exp, accum_out=s_bn[:]
            )
            ln_bn = smax.tile([T, 1], F32, tag="ln_bn")
            nc.scalar.activation(ln_bn[:], s_bn[:], AF.Ln)
            obn_st = smax.tile([T, 2], F32, tag="obn_st")
            nc.vector.tensor_scalar(
                obn_st[:], psum_bn[:], ln_bn[:], None, op0=ALU.subtract
            )
            nc.sync.dma_start(out=obn[:], in_=obn_st[:])

    nc.compile()
    _BUILD_CACHE[mode] = nc
    return nc


def _prep_inputs(inputs: dict, mode: str):
    np_dt1, np_dt2 = _np_dts(mode)

    fg_idx = np.asarray(inputs["fg_idx"])
    pool_feats = np.asarray(inputs["pool_feats"], dtype=np.float32)
    rnn_outs = np.asarray(inputs["rnn_outs"], dtype=np.float32)
    roi_labels = np.asarray(inputs["roi_labels"], dtype=np.float32)
    W_bn1 = np.asarray(inputs["W_bn1"], dtype=np.float32)
    b_bn1 = np.asarray(inputs["b_bn1"], dtype=np.float32)
    W_bn2 = np.asarray(inputs["W_bn2"], dtype=np.float32)
    b_bn2 = np.asarray(inputs["b_bn2"], dtype=np.float32)
    W_fg1 = np.asarray(inputs["W_fg1"], dtype=np.float32)
    b_fg1 = np.asarray(inputs["b_fg1"], dtype=np.float32)
    W_fg2 = np.asarray(inputs["W_fg2"], dtype=np.float32)
    b_fg2 = np.asarray(inputs["b_fg2"], dtype=np.float32)
    fg_emb = np.asarray(inputs["fg_emb"], dtype=np.float32)
    fg_mask = np.asarray(inputs["fg_mask"])

    # Shared (replicated) tensors. M order: fg hidden first, then bn.
    W1cat = np.concatenate([W_fg1, W_bn1], axis=0)          # (2048, 2048)
    W1T = np.ascontiguousarray(W1cat.T)                     # (2048 in, 2048 out)
    w1t = np.concatenate(
        [
            _pack_kmajor(
                np.ascontiguousarray(W1T[:, [mm * P + j for mm in ms for j in range(P)]])
            )
            for ms in CHUNK_MS
        ],
        axis=1,
    ).astype(np_dt1)
    W2p = np.zeros((FGP, RNN), np.float32)
    W2p[:300] = W_fg2
    w2t = _pack_kmajor(np.ascontiguousarray(W2p.T)).astype(np_dt2)
    w2bnt = _pack_kmajor(np.ascontiguousarray(W_bn2.T))
    sm2 = np.zeros((P, KT2 * 2 + 2), np.float32)
    sm2[:, 0 : KT2 * 2] = w2bnt
    sm2[0, KT2 * 2 :] = b_bn2
    sm2 = sm2.astype(np_dt2)
    embp = np.zeros((FGP, FG), np.float32)
    embp[:300] = fg_emb.T
    embt = _pack_kmajor(embp).astype(np_dt2)
    b1 = np.concatenate([b_fg1, b_bn1]).reshape(MT1, P).T
    bfg2p = np.zeros((FGP,), np.float32)
    bfg2p[:300] = b_fg2
    bfg2 = bfg2p.reshape(MT2, P).T
    smf = np.concatenate([b1, bfg2], axis=1).astype(np.float32)
    smf = np.ascontiguousarray(smf)

    # Per-token additive mask.
    maskadd_full = fg_mask[fg_idx].astype(np.float32) * np.float32(MIN_VALUE)

    cnt = roi_labels.sum(axis=2)
    cnt = np.where(cnt == 0, 1.0, cnt).astype(np.float32)
    roi_scaled = roi_labels / cnt[..., None]                # (B, S, R)

    in_maps = []
    for c in range(N_CORES):
        b0 = c * BPC
        pf = pool_feats[b0 : b0 + BPC]                      # (4, 128, 1024)
        poolf_c = np.ascontiguousarray(
            pf.transpose(1, 0, 2).reshape(P, BPC * RNN)
        ).astype(np_dt1)
        rt = roi_scaled[b0 : b0 + BPC].transpose(0, 2, 1)   # (4, 128, 16)
        roit_c = np.ascontiguousarray(
            rt.transpose(1, 0, 2).reshape(P, BPC * S)
        ).astype(np_dt1)
        xrT = rnn_outs[b0 : b0 + BPC].reshape(T, RNN).T     # (1024, 64)
        xr_c = _pack_kmajor(np.ascontiguousarray(xrT)).astype(np_dt1)
        xin_c = np.ascontiguousarray(
            np.concatenate([poolf_c, xr_c, roit_c], axis=1)
        )
        maskadd_c = np.ascontiguousarray(
            maskadd_full[b0 : b0 + BPC].reshape(T, FG)
        )
        in_maps.append(
            {
                "w1t": w1t,
                "w2t": w2t,
                "embt": embt,
                "xin": xin_c,
                "sm2": sm2,
                "smf": smf,
                "maskadd": maskadd_c,
            }
        )
    return in_maps


def kernel(**inputs):
    mode = COMPUTE_DTYPE
    nc = _build(mode)
    in_maps = _prep_inputs(inputs, mode)
    res = run_bass_kernel_spmd(nc, in_maps, list(range(N_CORES)))
    bn = np.stack(
        [res.results[c]["obn"].reshape(BPC, S, 2) for c in range(N_CORES)]
    ).reshape(B, S, 2)
    fg = np.stack(
        [res.results[c]["ofg"].reshape(BPC, S, FG) for c in range(N_CORES)]
    ).reshape(B, S, FG)
    return bn.astype(np.float32), fg.astype(np.float32)
